# revision 7
# baseline (speedup 1.0000x reference)
"""Trainium2 Bass kernel for the masked-retention attention module.

Strategy: data-parallel over batch (b=32 -> 4 per core x 8 cores).
Each core computes, per batch element:
  qkv projection (fp32r matmuls), S = q k^T and S^T per head,
  weights_before = softmax(S/64), mask = base + pattern*outer(ms),
  weights_after = softmax(S*mask/64), r = (S*mask/8) @ v + q @ pkv/8,
  GroupNorm over each head -> output.
Host side only does input marshalling (transpose/pad), the reference's
fixed-key PRNG noise for the mask (threefry - not implementable as a
plain tensor op), and the tiny data-independent mask constants.
"""

import functools
import math

import numpy as np

B, H, N, D = 32, 8, 257, 64
DIM = H * D          # 512
PIX = 256
EPS = 1e-5
NCORES = 8
B_LOC = B // NCORES  # 4
NE = 258             # even moving size for fp32r matmuls
NPAD = 264           # padded free dim of tiles / host arrays
CNT = float(N * D)   # groupnorm element count per head


# ---------------------------------------------------------------- host prep

@functools.lru_cache(maxsize=1)
def _noise():
    # reference uses jax.random.randint(key(42), [B,16,16], 1, 128) for the
    # zero-entries of msk; replicate it exactly on the CPU backend.
    import jax
    with jax.default_device(jax.devices("cpu")[0]):
        n = jax.random.randint(jax.random.key(42), (B, 16, 16), 1, 128,
                               dtype="int32")
        return np.asarray(n)


@functools.lru_cache(maxsize=1)
def _mask_consts():
    # mask = base + pattern * outer(ms), ms = m/mx per batch
    # base[0,0]=1; base[i,i]=1; base[i,0]=dec1[i] (i>=1); else 0
    # pattern[i,j]=1 iff i>=1, j>=1, j<i
    dec1 = np.exp(-5.0 * np.linspace(0.0, 1.0, N)).astype(np.float32)
    base = np.zeros((N, NPAD), np.float32)
    patt = np.zeros((N, NPAD), np.float32)
    idx = np.arange(N)
    base[idx, idx] = 1.0
    base[1:, 0] = dec1[1:]
    base[0, 0] = 1.0
    ii = idx[:, None]
    jj = idx[None, :]
    patt[:, :N] = ((ii >= 1) & (jj >= 1) & (jj < ii)).astype(np.float32)
    baseT8 = np.zeros((N, NPAD), np.float32)
    pattT8 = np.zeros((N, NPAD), np.float32)
    baseT8[:, :N] = base[:, :N].T / 8.0
    pattT8[:, :N] = patt[:, :N].T / 8.0
    return base, patt, baseT8, pattT8


def _host_prep(x, msk, W_qkv, past_kv, gn_weight, gn_bias):
    """Build the per-core input maps."""
    x = np.ascontiguousarray(x, np.float32)
    W_qkv = np.ascontiguousarray(W_qkv, np.float32)
    past_kv = np.ascontiguousarray(past_kv, np.float32)

    m = np.where(msk == 0, _noise(), msk).astype(np.float32).reshape(B, PIX)
    mx = m.max(axis=1, keepdims=True)
    ms = m / mx                                   # [B, 256]
    mrow = np.zeros((B, NPAD), np.float32)
    mrow[:, 1:1 + PIX] = ms                       # row/col index r uses mrow[r]

    xT = np.zeros((B, DIM, NPAD), np.float32)
    xT[:, :, :N] = x.transpose(0, 2, 1)

    wT = np.ascontiguousarray(W_qkv.T)            # [512, 1536]
    base, patt, baseT8, pattT8 = _mask_consts()

    in_maps = []
    for c in range(NCORES):
        sl = slice(c * B_LOC, (c + 1) * B_LOC)
        in_maps.append({
            "xT": np.ascontiguousarray(xT[sl]),
            "wT": wT,
            "pkv": np.ascontiguousarray(past_kv[sl]),
            "mrow": np.ascontiguousarray(mrow[sl]),
            "base_c": base, "patt_c": patt,
            "baseT8_c": baseT8, "pattT8_c": pattT8,
            "gnw": np.ascontiguousarray(gn_weight, np.float32),
            "gnb": np.ascontiguousarray(gn_bias, np.float32),
        })
    return in_maps


# ---------------------------------------------------------------- device

def build_nc(repeat=1):
    """Build and compile the per-core Bass program (B_LOC batches)."""
    import concourse.bacc as bacc
    import concourse.bass as bass
    import concourse.tile as tile
    from concourse import mybir

    f32 = mybir.dt.float32
    f32r = mybir.dt.float32r
    AF = mybir.ActivationFunctionType
    OP = mybir.AluOpType

    nc = bacc.Bacc("TRN2", target_bir_lowering=False, debug=False)

    xT = nc.dram_tensor("xT", [B_LOC, DIM, NPAD], f32, kind="ExternalInput").ap()
    wT = nc.dram_tensor("wT", [DIM, 3 * DIM], f32, kind="ExternalInput").ap()
    pkv = nc.dram_tensor("pkv", [B_LOC, H, D, D], f32, kind="ExternalInput").ap()
    mrow = nc.dram_tensor("mrow", [B_LOC, NPAD], f32, kind="ExternalInput").ap()
    base_c = nc.dram_tensor("base_c", [N, NPAD], f32, kind="ExternalInput").ap()
    patt_c = nc.dram_tensor("patt_c", [N, NPAD], f32, kind="ExternalInput").ap()
    baseT8_c = nc.dram_tensor("baseT8_c", [N, NPAD], f32, kind="ExternalInput").ap()
    pattT8_c = nc.dram_tensor("pattT8_c", [N, NPAD], f32, kind="ExternalInput").ap()
    gnw = nc.dram_tensor("gnw", [H], f32, kind="ExternalInput").ap()
    gnb = nc.dram_tensor("gnb", [H], f32, kind="ExternalInput").ap()

    out = nc.dram_tensor("out", [B_LOC, N, DIM], f32, kind="ExternalOutput").ap()
    wb = nc.dram_tensor("wb", [B_LOC, H, N, N], f32, kind="ExternalOutput").ap()
    wa = nc.dram_tensor("wa", [B_LOC, H, N, N], f32, kind="ExternalOutput").ap()

    def bcast(ap_row, parts):
        # replicate a 1-D dram row across `parts` partitions
        return bass.AP(tensor=ap_row.tensor, offset=ap_row.offset,
                       ap=[[0, parts]] + [list(p) for p in ap_row.ap])

    def col(ap_row, n_parts):
        # scatter a 1-D dram row down n_parts partitions, 1 elem each
        return bass.AP(tensor=ap_row.tensor, offset=ap_row.offset,
                       ap=[[1, n_parts], [0, 1]])

    def flat(ap):
        return ap.rearrange("p a b -> p (a b)")

    with tile.TileContext(nc) as tc, \
            tc.tile_pool(name="consts", bufs=1) as cst, \
            tc.tile_pool(name="wbounce", bufs=2) as wbp, \
            tc.tile_pool(name="batch", bufs=2) as bp, \
            tc.tile_pool(name="head", bufs=3) as hp, \
            tc.tile_pool(name="psS", bufs=1, space="PSUM") as psS, \
            tc.tile_pool(name="psR", bufs=2, space="PSUM") as psR:

        # ---- per-core constants
        wtr = cst.tile([128, 4, 3 * DIM], f32r)
        for f in range(4):
            wtmp = wbp.tile([128, 3 * DIM], f32)
            nc.sync.dma_start(wtmp[:], wT[128 * f:128 * (f + 1), :])
            nc.vector.tensor_copy(wtr[:, f, :], wtmp[:])

        patt_sb = cst.tile([128, 2, NPAD], f32)
        base_sb = cst.tile([128, 2, NPAD], f32)
        pattT8_sb = cst.tile([128, 2, NPAD], f32)
        baseT8_sb = cst.tile([128, 2, NPAD], f32)
        for sb, dr in ((patt_sb, patt_c), (base_sb, base_c),
                       (pattT8_sb, pattT8_c), (baseT8_sb, baseT8_c)):
            for c in range(2):
                nc.sync.dma_start(sb[:, c, :], dr[128 * c:128 * (c + 1), :])
        patt256_rep = cst.tile([H, NPAD], f32)
        base256_rep = cst.tile([H, NPAD], f32)
        nc.sync.dma_start(patt256_rep[:], bcast(patt_c[256, :], H))
        nc.sync.dma_start(base256_rep[:], bcast(base_c[256, :], H))

        gnw_sb = cst.tile([1, H], f32)
        gnb_sb = cst.tile([1, H], f32)
        nc.sync.dma_start(gnw_sb[:], gnw.rearrange("(o h) -> o h", o=1))
        nc.sync.dma_start(gnb_sb[:], gnb.rearrange("(o h) -> o h", o=1))

        ones128 = cst.tile([128, 1], f32)
        nc.vector.memset(ones128[:], 1.0)
        ones_row = cst.tile([1, 128], f32)
        nc.vector.memset(ones_row[:], 1.0)

        for _rep in range(repeat):
            for i in range(B_LOC):
                # ---------------- loads
                xt = bp.tile([128, 4, NPAD], f32)
                for f in range(4):
                    nc.sync.dma_start(xt[:, f, :], xT[i, 128 * f:128 * (f + 1), :])
                xtr = bp.tile([128, 4, NPAD], f32r)
                nc.vector.tensor_copy(xtr[:], xt[:])

                pkv_sb = bp.tile([64, H, D], f32)
                nc.sync.dma_start(pkv_sb[:], pkv[i].rearrange("h d e -> d h e"))
                pkvr = bp.tile([64, H, D], f32r)
                nc.gpsimd.tensor_scalar_mul(pkvr[:], pkv_sb[:], 0.125)

                mrow_b = bp.tile([128, NPAD], f32)
                nc.sync.dma_start(mrow_b[:], bcast(mrow[i, :], 128))
                mrow8 = bp.tile([H, NPAD], f32)
                nc.sync.dma_start(mrow8[:], bcast(mrow[i, :], H))
                mc = bp.tile([128, 2], f32)
                nc.sync.dma_start(mc[:, 0:1], col(mrow[i, 0:128], 128))
                nc.sync.dma_start(mc[:, 1:2], col(mrow[i, 128:256], 128))
                mc2 = bp.tile([H, 1], f32)
                nc.sync.dma_start(mc2[:], bcast(mrow[i, 256:257], H))

                # ---------------- projections
                qk_T = bp.tile([64, 16, NPAD], f32r)
                for oc in range(16):
                    ps_qk = psS.tile([64, NE], f32, tag="ps_proj", bufs=2)
                    for f in range(4):
                        nc.tensor.matmul(
                            ps_qk[:], wtr[:, f, 64 * oc:64 * (oc + 1)],
                            xtr[:, f, 0:NE], start=(f == 0), stop=(f == 3))
                    nc.scalar.copy(qk_T[:, oc, 0:NE], ps_qk[:])

                v0 = bp.tile([128, DIM], f32r)
                v1 = bp.tile([128, DIM], f32r)
                v2 = bp.tile([1, DIM], f32r)
                for ci, vt in ((0, v0), (1, v1), (2, v2)):
                    p = 128 if ci < 2 else 1
                    ps_v = psS.tile([128, DIM], f32, tag="ps_proj", bufs=2)
                    for f in range(4):
                        nc.tensor.matmul(
                            ps_v[0:p, :],
                            xtr[:, f, 128 * ci:128 * ci + p],
                            wtr[:, f, 2 * DIM:3 * DIM],
                            start=(f == 0), stop=(f == 3))
                    nc.scalar.copy(vt[:], ps_v[0:p, :])

                # blockdiag last-row builders + S[256,256] per head
                ql_bd = bp.tile([64, H, H], f32r)
                kl_bd = bp.tile([64, H, H], f32r)
                nc.vector.memset(ql_bd[:].bitcast(f32), 0.0)
                nc.vector.memset(kl_bd[:].bitcast(f32), 0.0)
                for h in range(H):
                    nc.vector.tensor_copy(ql_bd[:, h, h:h + 1],
                                          qk_T[:, h, 256:257])
                    nc.vector.tensor_copy(kl_bd[:, h, h:h + 1],
                                          qk_T[:, 8 + h, 256:257])
                dv = bp.tile([64, H], f32)
                nc.vector.tensor_mul(dv[:], qk_T[:, 0:8, 256],
                                     qk_T[:, 8:16, 256])
                sd_ps = psR.tile([1, H], f32, tag="rn")
                nc.tensor.matmul(sd_ps[:], ones128[0:64, :], dv[:])
                sdiag8 = bp.tile([1, H], f32r)
                nc.vector.tensor_scalar_mul(sdiag8[:], sd_ps[:], 0.125)

                # ---------------- mask build (gpsimd)
                msk_sb = bp.tile([128, 2, NPAD], f32)
                mskT8_sb = bp.tile([128, 2, NPAD], f32)
                for c in range(2):
                    nc.vector.scalar_tensor_tensor(
                        msk_sb[:, c, 0:NE], mrow_b[:, 0:NE], mc[:, c:c + 1],
                        patt_sb[:, c, 0:NE], op0=OP.mult, op1=OP.mult)
                    nc.gpsimd.tensor_add(
                        msk_sb[:, c, 0:NE], msk_sb[:, c, 0:NE],
                        base_sb[:, c, 0:NE])
                    nc.vector.scalar_tensor_tensor(
                        mskT8_sb[:, c, 0:NE], mrow_b[:, 0:NE], mc[:, c:c + 1],
                        pattT8_sb[:, c, 0:NE], op0=OP.mult, op1=OP.mult)
                    nc.gpsimd.tensor_add(
                        mskT8_sb[:, c, 0:NE], mskT8_sb[:, c, 0:NE],
                        baseT8_sb[:, c, 0:NE])
                mask_last = bp.tile([H, NPAD], f32)
                nc.vector.scalar_tensor_tensor(
                    mask_last[:, 0:NE], mrow8[:, 0:NE], mc2[:],
                    patt256_rep[:, 0:NE], op0=OP.mult, op1=OP.mult)
                nc.gpsimd.tensor_add(
                    mask_last[:, 0:NE], mask_last[:, 0:NE],
                    base256_rep[:, 0:NE])

                # ---------------- last rows via blockdiag matmuls
                s_last = psS.tile([H, NE], f32, tag="S0")
                t_last = psS.tile([H, NE], f32, tag="T0")
                for f in range(8):
                    nc.tensor.matmul(s_last[:], ql_bd[:, f, :],
                                     qk_T[:, 8 + f, 0:NE],
                                     start=(f == 0), stop=(f == 7))
                    nc.tensor.matmul(t_last[:], kl_bd[:, f, :],
                                     qk_T[:, f, 0:NE],
                                     start=(f == 0), stop=(f == 7))

                el_wb = bp.tile([H, NPAD], f32)
                el_wa = bp.tile([H, NPAD], f32)
                ma_last = bp.tile([H, NPAD], f32)
                sums_l = bp.tile([H, 4], f32)
                nc.scalar.activation(el_wb[:, 0:NE], s_last[:], AF.Exp,
                                     scale=1.0 / 64.0,
                                     accum_out=sums_l[:, 0:1])
                nc.vector.tensor_mul(ma_last[:, 0:NE], s_last[:],
                                     mask_last[:, 0:NE])
                nc.scalar.activation(el_wa[:, 0:NE], ma_last[:, 0:NE], AF.Exp,
                                     scale=1.0 / 64.0,
                                     accum_out=sums_l[:, 1:2])
                nc.vector.tensor_scalar_add(sums_l[:, 0:2], sums_l[:, 0:2], -1.0)
                nc.vector.reciprocal(sums_l[:, 2:4], sums_l[:, 0:2])
                nc.vector.tensor_scalar_mul(el_wb[:, 0:NE], el_wb[:, 0:NE],
                                            sums_l[:, 2:3])
                nc.vector.tensor_scalar_mul(el_wa[:, 0:NE], el_wa[:, 0:NE],
                                            sums_l[:, 3:4])
                nc.sync.dma_start(wb[i, :, 256, :], el_wb[:, 0:N])
                nc.sync.dma_start(wa[i, :, 256, :], el_wa[:, 0:N])

                # ---------------- per-head main work
                st = bp.tile([128, 2, H], f32)
                st2 = bp.tile([1, 2, H], f32)
                rn_sb = bp.tile([128, H, 3, D], f32)
                scr = bp.tile([128, 128], f32)

                for h in range(H):
                    qh = qk_T[:, h, :]
                    kh = qk_T[:, 8 + h, :]

                    S0 = psS.tile([128, NE], f32, tag="S0")
                    S1 = psS.tile([128, NE], f32, tag="S1")
                    T0 = psS.tile([128, NE], f32, tag="T0")
                    T1 = psS.tile([128, NE], f32, tag="T1")
                    nc.tensor.matmul(S0[:], qh[:, 0:128], kh[:, 0:NE])
                    nc.tensor.matmul(S1[:], qh[:, 128:256], kh[:, 0:NE])
                    nc.tensor.matmul(T0[:], kh[:, 0:128], qh[:, 0:NE])
                    nc.tensor.matmul(T1[:], kh[:, 128:256], qh[:, 0:NE])

                    e_wb = hp.tile([128, 2, NPAD], f32, tag="e_wb")
                    e_wa = hp.tile([128, 2, NPAD], f32, tag="e_wa")
                    ma = hp.tile([128, 2, NPAD], f32, tag="ma")
                    mmT8 = hp.tile([128, 2, NPAD], f32r, tag="mmT8")
                    sums = hp.tile([128, 8], f32, tag="sums")

                    for c, Sc, Tc in ((0, S0, T0), (1, S1, T1)):
                        nc.scalar.activation(e_wb[:, c, 0:NE], Sc[:], AF.Exp,
                                             scale=1.0 / 64.0,
                                             accum_out=sums[:, c:c + 1])
                        nc.vector.tensor_mul(ma[:, c, 0:NE], Sc[:],
                                             msk_sb[:, c, 0:NE])
                        nc.vector.tensor_mul(mmT8[:, c, 0:NE], Tc[:],
                                             mskT8_sb[:, c, 0:NE])
                        nc.scalar.activation(e_wa[:, c, 0:NE],
                                             ma[:, c, 0:NE], AF.Exp,
                                             scale=1.0 / 64.0,
                                             accum_out=sums[:, 2 + c:3 + c])
                    nc.vector.tensor_scalar_add(sums[:, 0:4], sums[:, 0:4],
                                                -1.0)
                    nc.vector.reciprocal(sums[:, 4:8], sums[:, 0:4])
                    for c in range(2):
                        nc.vector.tensor_scalar_mul(
                            e_wb[:, c, 0:NE], e_wb[:, c, 0:NE],
                            sums[:, 4 + c:5 + c])
                        nc.vector.tensor_scalar_mul(
                            e_wa[:, c, 0:NE], e_wa[:, c, 0:NE],
                            sums[:, 6 + c:7 + c])
                        nc.sync.dma_start(wb[i, h, 128 * c:128 * (c + 1), :],
                                          e_wb[:, c, 0:N])
                        nc.sync.dma_start(wa[i, h, 128 * c:128 * (c + 1), :],
                                          e_wa[:, c, 0:N])

                    # r = (S*mask/8) @ v + q @ pkv/8   in [n, d] orientation
                    rn = psR.tile([128, 3, D], f32, tag="rn")
                    for nci in range(3):
                        nsl = (slice(128 * nci, 128 * nci + (128 if nci < 2 else 1)))
                        o = rn[0:(128 if nci < 2 else 1), nci, :]
                        nc.tensor.matmul(o, mmT8[:, 0, nsl], v0[:, 64 * h:64 * h + 64],
                                         start=True, stop=False)
                        nc.tensor.matmul(o, mmT8[:, 1, nsl], v1[:, 64 * h:64 * h + 64],
                                         start=False, stop=False)
                        if nci == 2:
                            nc.tensor.matmul(o, sdiag8[:, h:h + 1],
                                             v2[:, 64 * h:64 * h + 64],
                                             start=False, stop=False)
                        nc.tensor.matmul(o, qh[:, nsl], pkvr[:, h, :],
                                         start=False, stop=True)

                    # stats + move r to sbuf
                    nc.scalar.activation(
                        rn_sb[:, h, 0:2, :].rearrange("p a b -> p (a b)"),
                        rn[:, 0:2, :].rearrange("p a b -> p (a b)"),
                        AF.Copy, accum_out=st[:, 0, h:h + 1])
                    nc.scalar.activation(
                        scr[:], rn[:, 0:2, :].rearrange("p a b -> p (a b)"),
                        AF.Square, accum_out=st[:, 1, h:h + 1])
                    nc.scalar.activation(rn_sb[0:1, h, 2, :], rn[0:1, 2, :],
                                         AF.Copy, accum_out=st2[:, 0, h:h + 1])
                    nc.scalar.activation(scr[0:1, 0:64], rn[0:1, 2, :],
                                         AF.Square, accum_out=st2[:, 1, h:h + 1])

                # ---------------- groupnorm finalize
                bst = psR.tile([1, 2, H], f32, tag="rn")
                nc.tensor.matmul(bst[:].rearrange("p a b -> p (a b)"),
                                 ones128[:],
                                 st[:].rearrange("p a b -> p (a b)"),
                                 start=True, stop=False)
                nc.tensor.matmul(bst[:].rearrange("p a b -> p (a b)"),
                                 ones128[0:1, :],
                                 st2[:].rearrange("p a b -> p (a b)"),
                                 start=False, stop=True)

                ab = bp.tile([1, 2, H], f32)
                t_m = bp.tile([1, 4, H], f32)
                nc.vector.tensor_scalar_mul(t_m[:, 0, :], bst[:, 0, :], 1.0 / CNT)
                nc.vector.tensor_scalar_mul(t_m[:, 1, :], bst[:, 1, :], 1.0 / CNT)
                nc.vector.tensor_mul(t_m[:, 2, :], t_m[:, 0, :], t_m[:, 0, :])
                nc.vector.tensor_sub(t_m[:, 3, :], t_m[:, 1, :], t_m[:, 2, :])
                nc.vector.tensor_scalar_add(t_m[:, 3, :], t_m[:, 3, :], EPS)
                nc.scalar.activation(t_m[:, 3, :], t_m[:, 3, :], AF.Ln)
                nc.scalar.activation(t_m[:, 3, :], t_m[:, 3, :], AF.Exp,
                                     scale=-0.5)
                nc.vector.tensor_mul(ab[:, 0, :], t_m[:, 3, :], gnw_sb[:])
                nc.vector.tensor_mul(t_m[:, 1, :], t_m[:, 0, :], ab[:, 0, :])
                nc.vector.tensor_sub(ab[:, 1, :], gnb_sb[:], t_m[:, 1, :])

                ab_bc = psR.tile([128, 2, H], f32, tag="rn")
                nc.tensor.matmul(ab_bc[:].rearrange("p a b -> p (a b)"),
                                 ones_row[:],
                                 ab[:].rearrange("p a b -> p (a b)"))
                ab_sb = bp.tile([128, 2, H], f32)
                nc.vector.tensor_copy(ab_sb[:], ab_bc[:])

                oc0 = bp.tile([128, DIM], f32)
                oc1 = bp.tile([128, DIM], f32)
                oc2 = bp.tile([1, DIM], f32)
                for h in range(H):
                    for c, ot in ((0, oc0), (1, oc1), (2, oc2)):
                        p = 128 if c < 2 else 1
                        nc.gpsimd.tensor_scalar(
                            ot[:, 64 * h:64 * (h + 1)],
                            rn_sb[0:p, h, c, :],
                            ab_sb[0:p, 0, h:h + 1], ab_sb[0:p, 1, h:h + 1],
                            op0=OP.mult, op1=OP.add)
                nc.sync.dma_start(out[i, 0:128, :], oc0[:])
                nc.sync.dma_start(out[i, 128:256, :], oc1[:])
                nc.sync.dma_start(out[i, 256:257, :], oc2[:])

    nc.compile()
    return nc


@functools.lru_cache(maxsize=2)
def _compiled_nc(repeat=1):
    return build_nc(repeat)


# ---------------------------------------------------------------- entry

def kernel(x, msk, W_qkv, past_kv, gn_weight, gn_bias):
    from concourse.bass_utils import run_bass_kernel_spmd

    in_maps = _host_prep(x, msk, W_qkv, past_kv, gn_weight, gn_bias)
    nc = _compiled_nc()
    res = run_bass_kernel_spmd(nc, in_maps, core_ids=list(range(NCORES)))

    output = np.empty((B, N, DIM), np.float32)
    weights_before = np.empty((B, H, N, N), np.float32)
    weights_after = np.empty((B, H, N, N), np.float32)
    for c in range(NCORES):
        sl = slice(c * B_LOC, (c + 1) * B_LOC)
        output[sl] = res.results[c]["out"]
        weights_before[sl] = res.results[c]["wb"]
        weights_after[sl] = res.results[c]["wa"]
    return output, weights_before, weights_after
